# revision 20
# baseline (speedup 1.0000x reference)
"""MoE experts kernel for Trainium2 (8 NeuronCores, expert-parallel).

Problem (nn_MoEExperts): T=2048 tokens, H=768 hidden, E=8 experts,
F=2048 ffn dim, top-2 routing.

    out[t] = sum_e cw[t,e] * ( gelu(x[t] @ w1[e].T) * (x[t] @ v1[e].T) ) @ w2[e]

Sharding: expert-parallel - core e holds expert e's three weight matrices
(each streamed from HBM exactly once).  Token dispatch by top_experts
happens host-side: tokens routed to expert e are gathered (pre-transposed)
into that core's input, padded to a common capacity C so all 8 cores run
one SPMD program.  The combine (scale by routing weight + scatter-add over
experts) happens host-side on the 8 returned per-expert outputs.

Matmul operands are fp16 (fp32 PSUM accumulation; ~5e-4 relative error,
full-rate 1 cycle/row on the tensor engine).  fp8 DoubleRow was measured
at only 2x fp16 per contraction on TRN2 hardware, which makes any
accuracy-preserving two-term fp8 scheme 1.5x SLOWER than fp16 - so fp16
everywhere is the optimal precision here (PE-bound kernel).

Device program per core:
  phase 1:  h1T = W1 @ xT, h2T = V1 @ xT   ([F, C] tiles, K=H, PSUM accum)
            gluT = gelu(h1T) * h2T         (ACT exact-erf Gelu + DVE mul)
  phase 2:  outT = H2 @ gluT = W2.T @ gluT ([H, C], K=F)

Startup is latency-tuned: the very first matmul group only needs w1[f0]
(sync queue, alone) and xt (split across the gpsimd and vector HWDGE
queues in consumption order; the scalar queue is delayed ~1.2us by the
Gelu ACT_TABLE_LOADs).  Warmup matmuls keep the PE busy from t~0.3us so
the HAM clock ramp completes before real work.  (The NEFF's fixed
zero-all-semaphores epilogue, ~250 serialized ops / ~9us, is compiler
boilerplate: --max-sem-num was A/B-tested to have no effect on it.)
"""

import os
import sys

if "/opt/trn_rl_repo" not in sys.path:
    sys.path.insert(0, "/opt/trn_rl_repo")

import numpy as np

E = 8
F = 2048
H = 768
TOPK = 2
P = 128
FT = F // P   # 16
KT = H // P   # 6
HT = H // P   # 6
# f-tiles per weight slab: singles early (fine-grained deps for the
# startup transient), wider once the pipeline is ahead.  sum = 16.
WV_SLABS = [1, 1, 1, 1, 4, 4, 4]
N_WARMUP = 10   # 8 x 512-col + 2 x 128-col

# Set by kernel() when KERNEL_TRACE=1.
LAST_EXEC_NS = None
LAST_MEAN_EXEC_NS = None
LAST_RESULTS = None


def _chunks(c):
    """Split c columns into moving-dim chunks <=512 (and >=256 when
    possible, so matmuls keep full rate)."""
    out = []
    rem = c
    while rem > 512:
        take = rem - 256 if (rem - 512 < 256 and rem < 1024) else 512
        out.append(take)
        rem -= take
    out.append(rem)
    return out


def _install_trace_shim():
    """Register the axon NTFF profile hook (antenv.axon_hooks is missing in
    this image) and neuter the remote artifact upload."""
    import types

    try:
        import antenv.axon_hooks  # noqa: F401
    except ImportError:
        mod = types.ModuleType("antenv.axon_hooks")
        mod._hook = None
        mod.set_axon_ntff_profile_hook = lambda h: setattr(mod, "_hook", h)
        mod.get_axon_ntff_profile_hook = lambda: mod._hook
        sys.modules["antenv.axon_hooks"] = mod
        import antenv

        antenv.axon_hooks = mod
        from trn_agent_boot.trn_boot import _ntff_profile_via_ctypes

        hook = _ntff_profile_via_ctypes("/opt/axon/libaxon_pjrt.so")
        if hook is not None:
            mod.set_axon_ntff_profile_hook(hook)

    import concourse.bass_utils as bu

    bu.upload_artifacts = lambda tmpdir: "local://skipped"


def _build_program(C):
    """SPMD Bass program for per-expert capacity C (multiple of 128)."""
    import concourse.mybir as mybir
    import concourse.tile as tile
    from concourse import bacc

    f32 = mybir.dt.float32
    mdt = mybir.dt.float16
    cch = _chunks(C)

    nc = bacc.Bacc(None, target_bir_lowering=False, debug=False)

    # Host-prepared layouts (partition index first, rows contiguous):
    #   xt [128p, KT, C]            xt[p,k,c]    = x[ids[c], k*128+p]
    #   wv [128p, FT, 2, KT, 128f]  wv[p,f,j,k,q]= Wj[f*128+q, k*128+p]
    #   w2 [128p, FT, H]            w2[p,s,h]    = W2[s*128+p, h]
    xt_d = nc.declare_dram_parameter("xt", [P, KT, C], mdt, isOutput=False)
    wv_d = nc.declare_dram_parameter("wv", [P, FT, 2, KT, P], mdt, isOutput=False)
    w2_d = nc.declare_dram_parameter("w2", [P, FT, H], mdt, isOutput=False)
    out_d = nc.declare_dram_parameter("out", [H, C], mdt, isOutput=True)

    with tile.TileContext(nc) as tc:
        with tc.tile_pool(name="persist", bufs=1) as persist, \
             tc.tile_pool(name="osb", bufs=4) as osb_pool, \
             tc.tile_pool(name="gtmp", bufs=3) as gtmp, \
             tc.tile_pool(name="ps1", bufs=2, space="PSUM") as ps1, \
             tc.tile_pool(name="ps2", bufs=4, space="PSUM") as ps2:

            # Startup DMAs are latency-tuned for the first matmul groups.
            # w1 streams on the sync queue, v1 on the scalar queue (their
            # consumption alternates h1/h2 per f-tile), xt split across
            # both.  The scalar queue starts ~1.2us late (behind the Gelu
            # ACT_TABLE_LOADs), which the order below accounts for.
            xt_sb = persist.tile([P, KT, C], mdt, tag="xt", name="xt_sb")
            slabs = []   # (w1 tile, v1 tile, first f, nf)
            wv_sb = []   # per f-tile: (w1 tile, v1 tile, index in slab)
            f0 = 0
            for b, nf in enumerate(WV_SLABS):
                tw = persist.tile([P, nf, KT, P], mdt, tag=f"w1s{b}",
                                  name=f"w1s{b}")
                tv = persist.tile([P, nf, KT, P], mdt, tag=f"v1s{b}",
                                  name=f"v1s{b}")
                slabs.append((tw, tv, f0, nf))
                for fi in range(nf):
                    wv_sb.append((tw, tv, fi))
                f0 += nf

            # sync: w1[f0] then xt[k0:2] gate the very first group.  The
            # scalar queue starts ~1.3us late (one ACT table load is always
            # hoisted above its first DMA), so sync carries the bigger
            # early share.
            nc.sync.dma_start(out=slabs[0][0], in_=wv_d.ap()[:, 0:1, 0])
            nc.sync.dma_start(out=xt_sb[:, 0:2], in_=xt_d.ap()[:, 0:2])
            nc.scalar.dma_start(out=xt_sb[:, 3:4], in_=xt_d.ap()[:, 3:4])
            nc.scalar.dma_start(out=xt_sb[:, 4:5], in_=xt_d.ap()[:, 4:5])
            nc.scalar.dma_start(out=xt_sb[:, 5:6], in_=xt_d.ap()[:, 5:6])
            nc.sync.dma_start(out=xt_sb[:, 2:3], in_=xt_d.ap()[:, 2:3])
            nc.sync.dma_start(out=slabs[0][1], in_=wv_d.ap()[:, 0:1, 1])
            # Remaining slabs: w1 on sync, v1 on scalar, in f order.
            for tw, tv, fs, nf in slabs[1:]:
                nc.sync.dma_start(out=tw, in_=wv_d.ap()[:, fs:fs + nf, 0])
                nc.scalar.dma_start(out=tv, in_=wv_d.ap()[:, fs:fs + nf, 1])

            # w2 is only needed in phase 2 -- queue it after the phase-1 weights
            w2_sb = persist.tile([P, FT, H], mdt, tag="w2", name="w2_sb")
            nc.sync.dma_start(out=w2_sb, in_=w2_d.ap())

            # Pre-warm the PE (HAM clock gate) with throwaway matmuls while
            # the first input DMAs are in flight: by the time real data
            # lands, the PE clock ramp is already under way.  Mostly large
            # tiles for sustained busy, small ones at the end so the queue
            # drains quickly when real work arrives.
            dummy = gtmp.tile([P, 512], mdt, tag="dummy", name="dummy")
            nc.gpsimd.memset(dummy, 0.0)
            for wi in range(N_WARMUP):
                d_ps = ps2.tile([P, 512], f32, tag="ops", name=f"warm{wi}")
                cols = 512 if wi < 8 else P
                nc.tensor.matmul(d_ps[:, :cols], dummy[:, :P], dummy[:, :cols],
                                 start=True, stop=True)

            glu_sb = persist.tile([P, FT, C], mdt, tag="glu", name="glu_sb")

            # ---- phase 1: gluT[F, C] = gelu(W1 @ xT) * (V1 @ xT) ----
            for f in range(FT):
                bw, bv, fi = wv_sb[f]
                col = 0
                for ch in cch:
                    h1 = ps1.tile([P, ch], f32, tag="h1", name=f"h1_{f}_{col}")
                    h2 = ps1.tile([P, ch], f32, tag="h2", name=f"h2_{f}_{col}")
                    for k in range(KT):
                        nc.tensor.matmul(h1[:], bw[:, fi, k, :],
                                         xt_sb[:, k, col:col + ch],
                                         start=(k == 0), stop=(k == KT - 1))
                    for k in range(KT):
                        nc.tensor.matmul(h2[:], bv[:, fi, k, :],
                                         xt_sb[:, k, col:col + ch],
                                         start=(k == 0), stop=(k == KT - 1))
                    g1 = gtmp.tile([P, ch], f32, tag="g1", name=f"g1_{f}_{col}")
                    nc.scalar.activation(g1[:], h1[:],
                                         mybir.ActivationFunctionType.Gelu)
                    nc.vector.tensor_mul(glu_sb[:, f, col:col + ch], g1[:], h2[:])
                    col += ch

            # ---- phase 2: outT[H, C] = W2.T @ gluT ----
            for h in range(HT):
                col = 0
                cch_h = cch
                if h >= HT - 2:
                    cch_h = []
                    for ch in cch:
                        if ch > 256:
                            cch_h += [ch - ch // 2, ch // 2]
                        else:
                            cch_h.append(ch)
                for ch in cch_h:
                    o_ps = ps2.tile([P, ch], f32, tag="ops", name=f"o_{h}_{col}")
                    for k in range(FT):
                        nc.tensor.matmul(o_ps[:],
                                         w2_sb[:, k, h * P:(h + 1) * P],
                                         glu_sb[:, k, col:col + ch],
                                         start=(k == 0), stop=(k == FT - 1))
                    o_sb = osb_pool.tile([P, ch], mdt, tag="osb",
                                         name=f"os_{h}_{col}")
                    eng = nc.sync if (h + col // 256) % 2 == 0 else nc.scalar
                    nc.vector.tensor_copy(o_sb[:], o_ps[:])
                    if h == HT - 1:
                        # final h-tile: split each chunk across both HWDGE
                        # queues so the end-of-kernel DMA drain is halved
                        half = ch // 2
                        nc.sync.dma_start(
                            out=out_d.ap()[h * P:(h + 1) * P, col:col + half],
                            in_=o_sb[:, :half])
                        nc.scalar.dma_start(
                            out=out_d.ap()[h * P:(h + 1) * P,
                                           col + half:col + ch],
                            in_=o_sb[:, half:ch])
                    else:
                        eng.dma_start(
                            out=out_d.ap()[h * P:(h + 1) * P, col:col + ch],
                            in_=o_sb[:])
                    col += ch

    nc.compile()
    return nc


def kernel(x, top_weights, w1, v1, w2, top_experts):
    global LAST_EXEC_NS, LAST_MEAN_EXEC_NS, LAST_RESULTS

    from concourse.bass_utils import run_bass_kernel_spmd

    npdt = np.float16

    x = np.asarray(x)
    bsz, q_len, hidden = x.shape
    T = bsz * q_len
    x2 = np.ascontiguousarray(x.reshape(T, hidden).astype(np.float32, copy=False))
    te = np.asarray(top_experts).astype(np.int64, copy=False)
    tw = np.asarray(top_weights).astype(np.float32, copy=False)
    w1r = np.asarray(w1, dtype=np.float32).reshape(E, F, H)
    v1r = np.asarray(v1, dtype=np.float32).reshape(E, F, H)
    w2r = np.asarray(w2, dtype=np.float32).reshape(E, F, H)

    # Host-side dispatch: combine weights per (token, expert) summed over
    # top-k slots (handles duplicate experts within a token's top-k).
    cw = np.zeros((T, E), np.float32)
    rows = np.repeat(np.arange(T), TOPK)
    np.add.at(cw, (rows, te.reshape(-1)), tw.reshape(-1))

    ids = [np.nonzero((te == e).any(axis=1))[0] for e in range(E)]
    counts = [len(i) for i in ids]
    C = max(256, -(-max(counts) // P) * P)

    in_maps = []
    for e in range(E):
        xg = np.zeros((C, H), npdt)
        ce = counts[e]
        if ce:
            xg[:ce] = x2[ids[e]].astype(npdt)
        # xt[p, k, c] = xg[c, k*128+p]
        xt = np.ascontiguousarray(xg.reshape(C, KT, P).transpose(2, 1, 0))
        # wv[p, f, j, k, q] = Wj[e][f*128+q, k*128+p]
        w1t = w1r[e].astype(npdt).reshape(FT, P, KT, P).transpose(3, 0, 2, 1)
        v1t = v1r[e].astype(npdt).reshape(FT, P, KT, P).transpose(3, 0, 2, 1)
        wv = np.ascontiguousarray(np.stack([w1t, v1t], axis=2))
        # w2h[p, s, h] = W2[e][s*128+p, h]
        w2h = np.ascontiguousarray(
            w2r[e].astype(npdt).reshape(FT, P, H).transpose(1, 0, 2))
        in_maps.append({"xt": xt, "wv": wv, "w2": w2h})

    nc = _build_program(C)

    trace = os.environ.get("KERNEL_TRACE", "") == "1"
    if trace:
        _install_trace_shim()
        res = run_bass_kernel_spmd(nc, in_maps, list(range(E)),
                                   trace=True, trace_cores=list(range(E)))
        LAST_EXEC_NS = res.exec_time_ns
        LAST_MEAN_EXEC_NS = res.mean_exec_time_ns
        LAST_RESULTS = res
    else:
        res = run_bass_kernel_spmd(nc, in_maps, list(range(E)))

    # Host-side combine: scale each expert's rows by its routing weight and
    # scatter-add back to token order.
    out = np.zeros((T, H), np.float32)
    for e in range(E):
        ce = counts[e]
        if not ce:
            continue
        oe = res.results[e]["out"][:, :ce].T.astype(np.float32)  # [ce, H]
        out[ids[e]] += oe * cw[ids[e], e][:, None]

    return out.reshape(bsz, q_len, hidden).astype(np.float32, copy=False)
